# revision 13
# baseline (speedup 1.0000x reference)
"""Trainium2 Bass kernel: block 8x8 2D-DCT + channel-pack + 8x nearest upsample.

Computes, for input x (8, 3, 256, 256) f32:
  out[b, 64c+8a+d, 8i+r, 8j+q] = sum_{m,n} D[a,m] x[b,c,8i+m,8j+n] D[d,n]
i.e. the reference nn_DCT2D: per-8x8-block orthonormal DCT-II, 64 coeffs packed
into channels, then 8x8 nearest-neighbor upsample back to (256, 256).

Strategy (pure data-parallel over batch, one core per batch element):
  - Step 1 (TensorE): A2 = X^T @ M'', the row-DCT over H, where M'' is the
    block-diagonal DCT factor with columns permuted to c'' = ie*128+16a+ip
    (i = 2*ip + ie). Output A2[kh] [128 x 256] for the two n_img halves.
  - Step 2 (TensorE): for each output channel phase d and row-parity ie,
    psum = A2[:, ie-half]^T @ R8[d], applying the second DCT factor; the
    constant R8 also performs the 8x W-upsample (columns ww = 8j+q).
  - Copy (DVE/ACT): broadcast-copy psum 8x along the free dim, materializing
    the 8x H-replication, into per-(c,d) tiles o4 [128 x 4096] with
    partition p = 16a+ip and free f = ie*2048 + r*256 + ww.
  - DMA out (both HWDGE rings): one 2 MB, 128-partition DMA per (c, d) with
    16 KB descriptors: partition (a, ip) -> channel 64c+8a+d rows
    [16ip, 16ip+16), which is contiguous in HBM.

Everything is f32; matmul accumulation in PSUM f32.
"""

import numpy as np

import concourse.bacc as bacc
import concourse.mybir as mybir
from concourse.tile import TileContext
from concourse.bass_utils import run_bass_kernel_spmd

N_CORES = 8
B, C, H, W = 8, 3, 256, 256
BS = 8          # DCT block size
F32 = mybir.dt.float32


def _dct_matrix() -> np.ndarray:
    n = np.arange(BS, dtype=np.float64)
    k = n[:, None]
    D = np.cos(np.pi * (2.0 * n[None, :] + 1.0) * k / (2.0 * BS))
    scale = np.full((BS,), np.sqrt(2.0 / BS))
    scale[0] = np.sqrt(1.0 / BS)
    return (D * scale[:, None]).astype(np.float32)


def _build_consts() -> tuple[np.ndarray, np.ndarray]:
    D = _dct_matrix()
    # M'' [2, 128, 256]: col c'' = ie*128 + 8*ip + a maps to DCT row
    # 32a + i with i = 2*ip + ie:  M''[k, c''] = D[a, k%8] iff k//8 == i.
    # (ip-major partition order so the output DMA's outer dst dim has 16
    # entries -> descriptors spread over all 16 SDMA engines.)
    Mpp = np.zeros((256, 256), np.float32)
    for k in range(256):
        i = k // 8
        for a in range(8):
            Mpp[k, (i % 2) * 128 + 8 * (i // 2) + a] = D[a, k % 8]
    M = np.stack([Mpp[:128], Mpp[128:]])

    # R8 [8, 2, 128, 256]: R8[d, kh, k', ww] = D[d, k'%8] iff
    # k'//8 == ww//8 - 16*kh  (second DCT factor + 8x W-upsample).
    R8 = np.zeros((8, 2, 128, 256), np.float32)
    kp = np.arange(128)
    for d in range(8):
        for kh in range(2):
            for ww in range(256):
                j = ww // 8
                if 16 * kh <= j < 16 * kh + 16:
                    rows = kp[kp // 8 == j - 16 * kh]
                    R8[d, kh, rows, ww] = D[d, rows % 8]
    return M, R8


def _build_module():
    nc = bacc.Bacc("TRN2", target_bir_lowering=False, debug=False,
                   enable_asserts=False)

    x_t = nc.dram_tensor("x", [C, H, W], F32, kind="ExternalInput")
    m_t = nc.dram_tensor("m", [2, 128, 256], F32, kind="ExternalInput")
    r_t = nc.dram_tensor("r", [8, 2, 128, 256], F32, kind="ExternalInput")
    out_t = nc.dram_tensor("out", [C * 64, H, W], F32, kind="ExternalOutput")
    # view with channel split into (c, a, d) for the output APs
    out_r = out_t.rearrange("(c a d) h w -> c a d h w", c=C, a=8, d=8)

    with TileContext(nc) as tc:
        with (
            tc.tile_pool(name="consts", bufs=1) as cpool,
            tc.tile_pool(name="xp", bufs=4) as xpool,
            tc.tile_pool(name="atp", bufs=4) as atpool,
            tc.tile_pool(name="outp", bufs=7) as opool,
            tc.tile_pool(name="psa", bufs=2, space="PSUM") as psa_pool,
            tc.tile_pool(name="ps2", bufs=6, space="PSUM") as ps2_pool,
        ):
            m_tiles = [cpool.tile_from(m_t[kt], name=f"m{kt}",
                                       forced_dma_engine=mybir.EngineType.Pool)
                       for kt in range(2)]
            r_tiles = [[cpool.tile_from(r_t[d, kh], name=f"r{d}{kh}",
                                        forced_dma_engine=mybir.EngineType.Pool)
                        for kh in range(2)]
                       for d in range(8)]

            for c in range(C):
                # load image c as two [128, 256] row tiles
                xt = []
                for kt in range(2):
                    tile = xpool.tile([128, 256], F32, tag="x")
                    nc.gpsimd.dma_start(out=tile[:, :],
                                        in_=x_t[c, kt * 128:(kt + 1) * 128, :])
                    xt.append(tile)

                # step 1: A2[kh] [n_img-half, c''=(ie, a, ip)]
                at = []
                for kh in range(2):
                    ps_a = psa_pool.tile([128, 256], F32, tag="psa")
                    for kt in range(2):
                        nc.tensor.matmul(
                            ps_a[:, :],
                            lhsT=xt[kt][:, kh * 128:(kh + 1) * 128],
                            rhs=m_tiles[kt][:, :],
                            start=(kt == 0), stop=(kt == 1),
                        )
                    a_sb = atpool.tile([128, 256], F32, tag="at")
                    nc.vector.tensor_copy(out=a_sb[:, :], in_=ps_a[:, :])
                    at.append(a_sb)

                # step 2 + H-replication, one [128, 4096] tile per (c, d)
                for d in range(8):
                    o4 = opool.tile([128, 4096], F32, tag="o4")
                    for ie in range(2):
                        ps2 = ps2_pool.tile([128, 256], F32, tag="ps2")
                        for kh in range(2):
                            nc.tensor.matmul(
                                ps2[:, :],
                                lhsT=at[kh][:, ie * 128:(ie + 1) * 128],
                                rhs=r_tiles[d][kh][:, :],
                                start=(kh == 0), stop=(kh == 1),
                            )
                        # copy + 8x duplicate -> f = ie*2048 + r*256 + ww
                        src_bc = ps2[:, None, :].to_broadcast([128, 8, 256])
                        dst = o4[:, ie * 2048:(ie + 1) * 2048].rearrange(
                            "p (rep w) -> p rep w", rep=8)
                        if (d + ie) % 2 == 0:
                            nc.vector.tensor_copy(out=dst, in_=src_bc)
                        else:
                            nc.scalar.copy(out=dst, in_=src_bc)

                    # one 2 MB DMA: partition (ip, a) -> rows [16ip, 16ip+16)
                    # of channel (c, a, d); 16 KB descriptors.
                    dst = out_r[c, :, d].rearrange(
                        "a (ip hh) w -> ip a (hh w)", hh=16)
                    eng = nc.sync if d % 2 == 0 else nc.scalar
                    eng.dma_start(out=dst, in_=o4[:, :])

    nc.compile()
    return nc


_CACHE: dict = {}


def _get_module():
    if "nc" not in _CACHE:
        _CACHE["nc"] = _build_module()
        _CACHE["consts"] = _build_consts()
    return _CACHE["nc"], _CACHE["consts"]


def kernel(x: np.ndarray) -> np.ndarray:
    x = np.ascontiguousarray(np.asarray(x, dtype=np.float32))
    assert x.shape == (B, C, H, W), x.shape

    nc, (M, R) = _get_module()
    in_maps = [{"x": x[b], "m": M, "r": R} for b in range(N_CORES)]
    res = run_bass_kernel_spmd(nc, in_maps, core_ids=list(range(N_CORES)))
    out = np.stack([res.results[b]["out"] for b in range(N_CORES)], axis=0)
    return out
